# revision 27
# baseline (speedup 1.0000x reference)
"""Trainium2 Bass kernel for nn_Decoder_1271310320240 (3-layer LSTM decoder).

Self-contained: builds a Bass/Tile program, shards the batch (1024 -> 8 x 128)
across 8 NeuronCores (data-parallel, weights replicated), runs SPMD via
bass_utils.run_bass_kernel_spmd, and reassembles the full output.

Per-core layout:
  - gates in PSUM as [B=128 partitions, 4d=1024] (two banks per layer),
    PyTorch gate column order [i, f, g, o].
  - matmuls out = lhsT.T @ rhs: lhsT = x^T / h^T slices [K=128, M=128] (fp16,
    stationary), rhs = W^T slices [K=128, N=512] (fp16, streaming). Bias via a
    K=1 matmul against a ones row.
  - ACT: sigmoid(i,f) 512-wide, tanh(g), sigmoid(o), tanh(c_new) per d-half.
  - DVE: fp16 elementwise, d-half-split tail; h fed back through 128x128 PE
    transposes (+ PSUM->SBUF copies) to rebuild h^T for the next matmuls.
  - h-phase (bias + W_hh) matmuls are emitted one layer-step early as PE
    filler under the elementwise tail.
  - All inputs host-packed into 3 DRAM tensors; a post-scheduling pass lowers
    multi-semaphore waits to single-wait NoOps (walrus encodes one wait per
    instruction).
"""

import sys
from contextlib import ExitStack

import numpy as np

sys.path.insert(0, "/opt/trn_rl_repo")

import concourse.bass as bass  # noqa: E402
import concourse.tile as tile  # noqa: E402
from concourse import mybir  # noqa: E402
from concourse.masks import make_identity  # noqa: E402

FP16 = mybir.dt.float16
F32 = mybir.dt.float32
AF = mybir.ActivationFunctionType

NL = 3
D = 256
BL = 128  # per-core batch
N_CORES = 8


TAU = 96


def build(tau=TAU, tail_split=2, tg_split=1, lower_waits=True, repeat=1,
          fuse_h=False, out16=True, tail="copy", ab="", wdt="bf16",
          gfirst=False):
    """Build the Bass module (single-core program, run SPMD on 8 cores).

    repeat>1 re-runs the whole tau loop (state re-initialized each round)
    for differential wall-clock timing; the graded path uses repeat=1.
    fuse_h: bias+W_hh matmuls stream N=1024 across both psum banks (3 MMs
    instead of 6). out16: DMA the fp16 h^T state directly (host undoes the
    transpose) instead of a f32 DVE copy per step.
    """
    ab = set(ab.split(",")) if ab else set()
    WDT = FP16 if wdt == "fp16" else mybir.dt.bfloat16
    nc = bass.Bass("TRN2", target_bir_lowering=False, debug=False)

    # all inputs pre-packed on host into 3 tensors -> 3 DMAs, so no
    # instruction accumulates too many semaphore waits
    wt = nc.dram_tensor("wt", [128, 12 * 1024], WDT, kind="ExternalInput")
    bs = nc.dram_tensor("bs", [1, 3 * 1024], WDT, kind="ExternalInput")
    st = nc.dram_tensor("st", [128, 6 * 256], FP16, kind="ExternalInput")
    outp = nc.dram_tensor("out", [tau, 128, 256],
                          FP16 if out16 else F32, kind="ExternalOutput")

    out_ap = outp.ap()

    with ExitStack() as ctx:
        tc = ctx.enter_context(tile.TileContext(nc))
        consts = ctx.enter_context(tc.tile_pool(name="consts", bufs=1))
        state = ctx.enter_context(tc.tile_pool(name="state", bufs=2))
        acts = ctx.enter_context(tc.tile_pool(name="acts", bufs=2))
        outs = ctx.enter_context(tc.tile_pool(name="outs", bufs=3))
        psum = ctx.enter_context(tc.tile_pool(name="psum", bufs=1, space="PSUM"))
        psumT = ctx.enter_context(tc.tile_pool(name="psumT", bufs=2, space="PSUM"))

        # ---- constants (single DMA each) ----
        wtile = consts.tile([128, 12 * 1024], WDT, tag="wtile")
        # one DMA per layer so step-0 matmuls of layer 0 start as soon as
        # its weight block lands instead of waiting for the full 3 MB
        for l in range(NL):
            sl_w = slice(l * 4096, (l + 1) * 4096)
            nc.sync.dma_start(out=wtile[:, sl_w], in_=wt.ap()[:, sl_w])
        w = [[[wtile[:, ((l * 2 + m) * 2 + k) * 1024:((l * 2 + m) * 2 + k + 1) * 1024]
               for k in range(2)] for m in range(2)] for l in range(NL)]
        bstile = consts.tile([1, 3 * 1024], WDT, tag="bstile")
        nc.sync.dma_start(out=bstile, in_=bs.ap())
        bst = [bstile[:, l * 1024:(l + 1) * 1024] for l in range(NL)]
        sttile = consts.tile([128, 6 * 256], FP16, tag="sttile")
        nc.sync.dma_start(out=sttile, in_=st.ap())
        ones_t = consts.tile([1, 128], FP16, tag="ones")
        nc.gpsimd.memset(ones_t, 1.0)
        ident = consts.tile([128, 128], FP16, tag="ident")
        make_identity(nc, ident)

        # ---- state (step 0: DVE copies out of sttile, so step-0 elementwise
        # consumers only ever wait on one engine semaphore — the DVE
        # TensorTensor encoding supports a single sync wait) ----
        hT = [None] * NL
        c = [None] * NL

        pending_ps = [None] * NL  # psum tile with bias+hh accumulated
        ps_readers = [None] * NL  # ACT instructions that read the psum banks

        def h_phase(l):
            """bias + W_hh matmuls for layer l (next use of its psum bank)."""
            if ps_readers[l] is not None:
                # PE nop absorbing the WAR-on-ACT wait so the bias matmul
                # below needs only its single PE-drain wait (the HW
                # instruction encoding allows one semaphore wait).
                nop = nc.engines[mybir.EngineType.PE].nop(
                    nofuse=True, hint=f"war_absorb{l}")
                for rd in ps_readers[l]:
                    tile.add_dep_helper(nop.ins, rd.ins, sync=True,
                                        reason="absorb psum WAR")
            if fuse_h:
                # one [128,1024] psum tile spanning two banks; bias and the
                # two W_hh matmuls each stream the full 1024 columns
                AB = psum.tile([128, 1024], F32, tag=f"psAB{l}",
                               name=f"psAB{l}")
                A, Bk = AB[:, 0:512], AB[:, 512:1024]
                nc.tensor.matmul(AB, ones_t, bst[l], start=True, stop=False)
                nc.tensor.matmul(AB, hT[l][:, 0:128], w[l][1][0],
                                 start=False, stop=False)
                nc.tensor.matmul(AB, hT[l][:, 128:256], w[l][1][1],
                                 start=False, stop=False)
            else:
                A = psum.tile([128, 512], F32, tag=f"psA{l}", name=f"psA{l}")
                Bk = psum.tile([128, 512], F32, tag=f"psB{l}", name=f"psB{l}")
                nc.tensor.matmul(A, ones_t, bst[l][:, 0:512],
                                 start=True, stop=False)
                nc.tensor.matmul(Bk, ones_t, bst[l][:, 512:1024],
                                 start=True, stop=False)
                if "noh" not in ab:
                    hw_ = 256 if "mm256" in ab else 512
                    # k-major order so consecutive matmuls share stationary
                    # h^T blocks (LDW dedupe pass drops the repeats)
                    nc.tensor.matmul(A[:, 0:hw_], hT[l][:, 0:128],
                                     w[l][1][0][:, 0:hw_],
                                     start=False, stop=False,
                                     skip_group_check=True)
                    nc.tensor.matmul(Bk[:, 0:hw_], hT[l][:, 0:128],
                                     w[l][1][0][:, 512:512 + hw_],
                                     start=False, stop=False,
                                     skip_group_check=True)
                    nc.tensor.matmul(A[:, 0:hw_], hT[l][:, 128:256],
                                     w[l][1][1][:, 0:hw_],
                                     start=False, stop=False,
                                     skip_group_check=True)
                    nc.tensor.matmul(Bk[:, 0:hw_], hT[l][:, 128:256],
                                     w[l][1][1][:, 512:512 + hw_],
                                     start=False, stop=False,
                                     skip_group_check=True)
            pending_ps[l] = (A, Bk)

        for rep in range(repeat):
            for l in range(NL):
                hT[l] = state.tile([128, 256], FP16, tag=f"hT{l}",
                                   name=f"hT{l}")
                c[l] = state.tile([128, 256], FP16, tag=f"c{l}",
                                  name=f"c{l}")
                nc.scalar.copy(hT[l], sttile[:, l * 256:(l + 1) * 256])
                nc.scalar.copy(c[l], sttile[:, 768 + l * 256:768 + (l + 1) * 256])

            # prologue: h-phases for step 0 layers 0 and 1
            h_phase(0)
            h_phase(1)

            for t in range(tau):
                for l in range(NL):
                    xT = hT[NL - 1] if l == 0 else hT[l - 1]
                    A, Bk = pending_ps[l]
                    # x-phase matmuls (critical path). i/f bank (A) first so
                    # sigmoid(i,f) starts as early as possible; its k0 matmul
                    # only needs the first transposed half of the previous h.
                    if gfirst and "nox" not in ab:
                        # g-gate matmuls first as their own N=256 group so
                        # tanh(g) runs on ACT before the 512-wide sigmoid;
                        # o-gate group last (only needed for h at the end)
                        nc.tensor.matmul(Bk[:, 0:256], xT[:, 0:128],
                                         w[l][0][0][:, 512:768],
                                         start=False, stop=False,
                                         skip_group_check=True)
                        nc.tensor.matmul(Bk[:, 0:256], xT[:, 128:256],
                                         w[l][0][1][:, 512:768],
                                         start=False, stop=True,
                                         skip_group_check=True)
                        nc.tensor.matmul(A, xT[:, 0:128],
                                         w[l][0][0][:, 0:512],
                                         start=False, stop=False,
                                         skip_group_check=True)
                        nc.tensor.matmul(A, xT[:, 128:256],
                                         w[l][0][1][:, 0:512],
                                         start=False, stop=True,
                                         skip_group_check=True)
                        nc.tensor.matmul(Bk[:, 256:512], xT[:, 0:128],
                                         w[l][0][0][:, 768:1024],
                                         start=False, stop=False,
                                         skip_group_check=True)
                        nc.tensor.matmul(Bk[:, 256:512], xT[:, 128:256],
                                         w[l][0][1][:, 768:1024],
                                         start=False, stop=True,
                                         skip_group_check=True)
                    elif "nox" not in ab:
                        xw_ = 256 if "mm256" in ab else 512
                        nc.tensor.matmul(A[:, 0:xw_], xT[:, 0:128],
                                         w[l][0][0][:, 0:xw_],
                                         start=False, stop=False,
                                         skip_group_check=True)
                        nc.tensor.matmul(A[:, 0:xw_], xT[:, 128:256],
                                         w[l][0][1][:, 0:xw_],
                                         start=False, stop=True,
                                         skip_group_check=True)
                        nc.tensor.matmul(Bk[:, 0:xw_], xT[:, 0:128],
                                         w[l][0][0][:, 512:512 + xw_],
                                         start=False, stop=False,
                                         skip_group_check=True)
                        nc.tensor.matmul(Bk[:, 0:xw_], xT[:, 128:256],
                                         w[l][0][1][:, 512:512 + xw_],
                                         start=False, stop=True,
                                         skip_group_check=True)
                    else:
                        nc.tensor.matmul(A, xT[0:1, 0:128],
                                         w[l][0][0][0:1, 0:512],
                                         start=False, stop=True,
                                         skip_group_check=True)
                        nc.tensor.matmul(Bk, xT[0:1, 0:128],
                                         w[l][0][0][0:1, 512:1024],
                                         start=False, stop=True,
                                         skip_group_check=True)

                    # PE filler: h-phase of the layer-step 2 ahead
                    nl_, nt_ = (l + 2) % NL, t + (l + 2) // NL
                    if nt_ < tau:
                        h_phase(nl_)

                    # ACT: gate nonlinearities (i: 0:256, f: 256:512, g, o)
                    sif = acts.tile([128, 512], FP16, tag="sif")
                    if gfirst and "notg" not in ab:
                        tg = acts.tile([128, 256], FP16, tag="tg")
                        i_tg = nc.scalar.activation(tg, Bk[:, 0:256], AF.Tanh)
                        i_sif = nc.scalar.activation(sif, A, AF.Sigmoid)
                        so = acts.tile([128, 256], FP16, tag="so")
                        i_so = nc.scalar.activation(so, Bk[:, 256:512],
                                                    AF.Sigmoid)
                        ps_readers[l] = [i_tg, i_sif, i_so]
                    elif "notg" not in ab:
                        i_sif = nc.scalar.activation(sif, A, AF.Sigmoid)
                        tg = acts.tile([128, 256], FP16, tag="tg")
                        i_tg = nc.scalar.activation(tg, Bk[:, 0:256], AF.Tanh)
                        so = acts.tile([128, 256], FP16, tag="so")
                        i_so = nc.scalar.activation(so, Bk[:, 256:512],
                                                    AF.Sigmoid)
                        ps_readers[l] = [i_sif, i_tg, i_so]
                    else:
                        i_sif = nc.scalar.activation(sif, A, AF.Sigmoid)
                        tg = sif[:, 0:256]
                        so = sif[:, 256:512]
                        ps_readers[l] = [i_sif]

                    # DVE: c_new = sig(f)*c + sig(i)*tanh(g)
                    cn = state.tile([128, 256], FP16, tag=f"c{l}", name=f"cn{l}")
                    hTn = state.tile([128, 256], FP16, tag=f"hT{l}",
                                     name=f"hTn{l}")
                    if "notail" in ab:
                        nc.vector.tensor_copy(hTn, sif[:, 0:256])
                        c[l] = c[l]  # c unchanged
                        hT[l] = hTn
                        if l == NL - 1:
                            if out16:
                                nc.sync.dma_start(out=out_ap[t], in_=hTn)
                        continue
                    if tail == "m1":
                        # full-width tail: 4 ACT + 5 DVE + 2 transposes
                        fc = acts.tile([128, 256], FP16, tag="fc")
                        nc.vector.tensor_mul(fc, sif[:, 256:512], c[l])
                        ig = acts.tile([128, 256], FP16, tag="ig")
                        nc.vector.tensor_mul(ig, sif[:, 0:256], tg)
                        nc.vector.tensor_add(cn, fc, ig)
                        tct = acts.tile([128, 256], FP16, tag="tc")
                        nc.scalar.activation(tct, cn, AF.Tanh)
                        h16 = acts.tile([128, 256], FP16, tag="h16")
                        nc.vector.tensor_mul(h16, so, tct)
                        pst = psumT.tile([128, 256], FP16, tag="pst",
                                         bufs=2)
                        nc.tensor.transpose(pst[:, 0:128], h16[:, 0:128],
                                            ident)
                        nc.tensor.transpose(pst[:, 128:256], h16[:, 128:256],
                                            ident)
                        nc.vector.tensor_copy(hTn, pst)
                    elif tail_split == 2 and tail == "copy":
                        # d-half-split tail: h16 = so*tct, PE-transpose h16,
                        # DVE copy psum->sbuf rebuilds h^T
                        h16 = acts.tile([128, 256], FP16, tag="h16")
                        fcs = []
                        for hf in range(2):
                            sl_ = slice(hf * 128, (hf + 1) * 128)
                            sl_f = slice(256 + hf * 128, 256 + (hf + 1) * 128)
                            fc = acts.tile([128, 128], FP16, tag=f"fc{hf}",
                                           name=f"fc{hf}")
                            nc.vector.tensor_mul(fc, sif[:, sl_f], c[l][:, sl_])
                            fcs.append(fc)
                        for hf in range(2):
                            sl_ = slice(hf * 128, (hf + 1) * 128)
                            ig = acts.tile([128, 128], FP16, tag=f"ig{hf}",
                                           name=f"ig{hf}")
                            nc.vector.tensor_mul(ig, sif[:, sl_], tg[:, sl_])
                            nc.vector.tensor_add(cn[:, sl_], fcs[hf], ig)
                            if "notct" not in ab:
                                tct = acts.tile([128, 128], FP16,
                                                tag=f"tc{hf}",
                                                name=f"tct{hf}")
                                nc.scalar.activation(tct, cn[:, sl_], AF.Tanh)
                            else:
                                tct = cn[:, sl_]
                            nc.vector.tensor_mul(h16[:, sl_], so[:, sl_], tct)
                            pst = psumT.tile([128, 128], FP16, tag=f"pst{hf}",
                                             name=f"pst{hf}", bufs=1)
                            if "notr" not in ab:
                                nc.tensor.transpose(pst, h16[:, sl_], ident)
                                nc.vector.tensor_copy(hTn[:, sl_], pst)
                            else:
                                nc.vector.tensor_copy(hTn[:, sl_],
                                                      h16[:, sl_])
                    elif tail_split == 2:
                        # d-half-split tail, h^T produced as soT (x) tctT:
                        # so and tct are PE-transposed; one DVE mul per half
                        # writes hTn directly (no h16 + psum->sbuf copy hop).
                        # DVE order: fc halves, ig/cn halves, soS copies,
                        # hT muls last (they gate on the tct transposes).
                        ptileh = [psumT.tile([128, 256], FP16,
                                              tag=f"tp{hf}", name=f"tp{hf}",
                                              bufs=1) for hf in range(2)]
                        psoT = [ptileh[0][:, 0:128], ptileh[1][:, 0:128]]
                        for hf in range(2):
                            sl_ = slice(hf * 128, (hf + 1) * 128)
                            nc.tensor.transpose(psoT[hf], so[:, sl_], ident)
                        fcs = []
                        for hf in range(2):
                            sl_ = slice(hf * 128, (hf + 1) * 128)
                            sl_f = slice(256 + hf * 128, 256 + (hf + 1) * 128)
                            fc = acts.tile([128, 128], FP16, tag=f"fc{hf}",
                                           name=f"fc{hf}")
                            nc.vector.tensor_mul(fc, sif[:, sl_f], c[l][:, sl_])
                            fcs.append(fc)
                        ptts = []
                        for hf in range(2):
                            sl_ = slice(hf * 128, (hf + 1) * 128)
                            ig = acts.tile([128, 128], FP16, tag=f"ig{hf}",
                                           name=f"ig{hf}")
                            nc.vector.tensor_mul(ig, sif[:, sl_], tg[:, sl_])
                            nc.vector.tensor_add(cn[:, sl_], fcs[hf], ig)
                            tct = acts.tile([128, 128], FP16, tag=f"tc{hf}",
                                            name=f"tct{hf}")
                            nc.scalar.activation(tct, cn[:, sl_], AF.Tanh)
                            ptt = ptileh[hf][:, 128:256]
                            nc.tensor.transpose(ptt, tct, ident)
                            ptts.append(ptt)
                        soSs = []
                        for hf in range(2):
                            soS = acts.tile([128, 128], FP16, tag=f"soS{hf}",
                                            name=f"soS{hf}")
                            nc.vector.tensor_copy(soS, psoT[hf])
                            soSs.append(soS)
                        for hf in range(2):
                            sl_ = slice(hf * 128, (hf + 1) * 128)
                            nc.vector.tensor_mul(hTn[:, sl_], soSs[hf], ptts[hf])
                    else:
                        h16 = acts.tile([128, 256], FP16, tag="h16")
                        fc = acts.tile([128, 256], FP16, tag="fc")
                        nc.vector.tensor_mul(fc, sif[:, 256:512], c[l])
                        ig = acts.tile([128, 256], FP16, tag="ig")
                        nc.vector.tensor_mul(ig, sif[:, 0:256], tg)
                        nc.vector.tensor_add(cn, fc, ig)
                        tct = acts.tile([128, 256], FP16, tag="tc")
                        nc.scalar.activation(tct, cn, AF.Tanh)
                        nc.vector.tensor_mul(h16, so, tct)
                        pst = psumT.tile([128, 256], FP16, tag="pst")
                        nc.tensor.transpose(pst[:, 0:128], h16[:, 0:128], ident)
                        nc.tensor.transpose(pst[:, 128:256], h16[:, 128:256],
                                            ident)
                        nc.vector.tensor_copy(hTn, pst)
                    c[l] = cn
                    hT[l] = hTn

                    if l == NL - 1:
                        if "nodma" in ab:
                            pass
                        elif out16:
                            # DMA the fp16 h^T tile directly; host undoes
                            # the transpose in assemble_output
                            nc.sync.dma_start(out=out_ap[t], in_=hTn)
                        else:
                            h32 = outs.tile([128, 256], F32, tag="h32")
                            nc.vector.tensor_copy(h32, h16)
                            nc.sync.dma_start(out=out_ap[t], in_=h32)

    if "nodedup" not in ab:
        _dedupe_ldweights(nc)
    if lower_waits:
        _enforce_single_wait(nc)
    return nc


def _dedupe_ldweights(nc):
    """Drop InstLdweights that reload the stationary operand already in
    the PE array (identical AP + flags as the previous Ldweights). The
    deleted instruction's sem waits/updates move onto the next kept
    instruction (the following matmul)."""
    import concourse.mybir as mb
    fn = nc.m.functions[0]
    for blk in fn.blocks:
        out = []
        prev_key = None
        carry_waits, carry_updates = [], []
        for ins in blk.instructions:
            eng = getattr(ins, "engine", None)
            if eng != mybir.EngineType.PE:
                out.append(ins)
                continue
            if isinstance(ins, mb.InstLdweights):
                key = (str(ins.ins[0]), str(getattr(ins, "is_transpose", None)),
                       str(getattr(ins, "perf_mode", None)))
                if key == prev_key:
                    si = ins.sync_info
                    if si is not None:
                        carry_waits.extend(si.on_wait)
                        carry_updates.extend(si.on_update)
                    continue  # drop the reload
                prev_key = key
            if carry_waits or carry_updates:
                si = ins.sync_info
                ws = list(si.on_wait) if si else []
                us = list(si.on_update) if si else []
                ins.sync_info = mb.SyncInfo(on_wait=ws + carry_waits,
                                            on_update=us + carry_updates)
                carry_waits, carry_updates = [], []
            out.append(ins)
        assert not carry_waits and not carry_updates
        blk.instructions = out


def _enforce_single_wait(nc):
    """Walrus only encodes ONE semaphore wait per compute instruction.

    The sequencer dispatches in order, so any wait on an earlier instruction
    of the same engine queue also gates every later instruction. Drop waits
    that are covered by earlier same-queue waits; the war_absorb nops emitted
    in the build guarantee coverage exists for the known 2-wait cases.
    """
    import concourse.mybir as mb
    fn = nc.m.functions[0]
    ctr = 0
    for blk in fn.blocks:
        cover = {}  # engine -> {sem_name: max value waited}
        out = []
        changed = False
        for ins in blk.instructions:
            si = ins.sync_info
            if si is not None and len(si.on_wait) > 1:
                eng = ins.engine
                cov = cover.setdefault(str(eng), {})
                kept = [w for w in si.on_wait
                        if not (w.wait_mode == "sem-ge-imm"
                                and cov.get(w.ant_name, -1) >= w.wait_value)]
                # extra waits become single-wait NoOps on the same queue
                for w in kept[:-1]:
                    ctr += 1
                    nop = mb.InstNoOp(
                        name=f"swx{ctr}", engine=eng,
                        sync_info=mb.SyncInfo(on_wait=[w], on_update=[]))
                    out.append(nop)
                    if w.wait_mode == "sem-ge-imm":
                        cov[w.ant_name] = max(cov.get(w.ant_name, -1),
                                              w.wait_value)
                ins.sync_info = mb.SyncInfo(on_wait=kept[-1:],
                                            on_update=list(si.on_update))
                changed = True
            si2 = ins.sync_info
            if si2 is not None and si2.on_wait:
                cov = cover.setdefault(str(getattr(ins, "engine", None)), {})
                for w in si2.on_wait:
                    if w.wait_mode == "sem-ge-imm":
                        cov[w.ant_name] = max(cov.get(w.ant_name, -1),
                                              w.wait_value)
            out.append(ins)
        if changed:
            blk.instructions = out


# ---------------- host-side pre/post-processing ----------------

def prep_inputs(hidden, cell, W_ih, W_hh, b_ih, b_hh, wdt="bf16"):
    """Full inputs -> list of 8 per-core input maps (numpy)."""
    hidden = np.asarray(hidden, np.float32)
    cell = np.asarray(cell, np.float32)
    W_ih = np.asarray(W_ih, np.float32)
    W_hh = np.asarray(W_hh, np.float32)
    b_ih = np.asarray(b_ih, np.float32)
    b_hh = np.asarray(b_hh, np.float32)

    # weights packed [128, 12*1024]: col block (l,m,k) holds W_m[l][:, k*128+p].T
    wstk = np.stack([W_ih, W_hh], axis=1)            # [l, m, 4d, d]
    wtr = wstk.transpose(0, 1, 3, 2)                 # [l, m, d, 4d]
    wtr = wtr.reshape(NL, 2, 2, 128, 1024)           # [l, m, k, p, col]
    if wdt == "bf16":
        import ml_dtypes
        wnp = ml_dtypes.bfloat16
    else:
        wnp = np.float16
    wt = wtr.transpose(3, 0, 1, 2, 4).reshape(128, 12 * 1024).astype(wnp)
    bs = (b_ih + b_hh).reshape(1, 3 * 1024).astype(wnp)

    in_maps = []
    for ci in range(N_CORES):
        sl = slice(ci * BL, (ci + 1) * BL)
        ht = hidden[:, sl, :].transpose(0, 2, 1)     # [l, d, b]
        ht = ht.reshape(NL, 2, 128, BL).transpose(2, 0, 1, 3).reshape(128, 768)
        cc = cell[:, sl, :].transpose(1, 0, 2).reshape(128, 768)  # [b, (l,d)]
        stt = np.concatenate([ht, cc], axis=1).astype(np.float16)
        in_maps.append({
            "wt": wt,
            "bs": bs,
            "st": np.ascontiguousarray(stt),
        })
    return in_maps


def assemble_output(results, tau=TAU):
    """list of per-core out tiles -> [1024, tau, 256] f32.

    fp16 out tiles hold h^T: out[t, p, k*128+b] = h[b, k*128+p]; f32 tiles
    hold h directly as [t, b, d].
    """
    full = np.empty((N_CORES * BL, tau, D), np.float32)
    for ci, r in enumerate(results):
        o = r["out"]
        if o.dtype == np.float16:
            ht = o.reshape(tau, 128, 2, BL)          # [t, p, k, b]
            full[ci * BL:(ci + 1) * BL] = (
                ht.transpose(3, 0, 2, 1).reshape(BL, tau, D).astype(np.float32))
        else:
            full[ci * BL:(ci + 1) * BL] = o.transpose(1, 0, 2)
    return full


_NC_CACHE = {}


def _get_nc(tau):
    if tau not in _NC_CACHE:
        _NC_CACHE[tau] = build(tau)
    return _NC_CACHE[tau]


def kernel(hidden, cell, W_ih, W_hh, b_ih, b_hh, tau):
    from concourse.bass_utils import run_bass_kernel_spmd

    tau = int(np.asarray(tau))
    nc = _get_nc(tau)  # program is built (and cached) for the requested tau
    in_maps = prep_inputs(hidden, cell, W_ih, W_hh, b_ih, b_hh)
    res = run_bass_kernel_spmd(nc, in_maps, core_ids=list(range(N_CORES)))
    return assemble_output(res.results, tau)
